# revision 1
# baseline (speedup 1.0000x reference)
"""Causal self-attention (RoPE) Trainium2 kernel, 8-core SPMD.

Sharding: core c -> (batch b = c//2, head-group g = c%2). Each core computes
8 heads x 1 batch of attention plus its slice of the QKV/output projections;
the host sums the two head-group partial outputs per batch.

Device layouts (T = feature-on-partitions):
  xT   [i=128-chunk, s]      bf16   (host pre-transposes x)
  qT,kT[o=128-chunk, s]      bf16   o rows are RoPE-de-interleaved per head
                                    (even dims rows 0-31, odd dims 32-63)
  vp   [s-part, sc, h, 65]   bf16   v packed per head with a ones column
                                    (col 64) so A@V' also yields softmax sums
  S^T  [keys, queries] PSUM         exp(scale*S^T) directly gives P^T for AV
  out-proj emits [s, o] so the DRAM store is contiguous

RoPE pairs are de-interleaved by permuting Wq/Wk rows on the host (even
dims in rows 0-31 of each head, odd in 32-63), so the rotation pair-swap
becomes four partition-block SBUF-to-SBUF DMA copies. cos/sin tables and
the causal triangle mask are host-built inputs; all inputs are shipped
pre-transposed/pre-cast to bf16. 1/sqrt(dk) is folded into the exp's
scale. Softmax skips max-subtraction (scores ~ N(0,1) after the 1/8
scale; no overflow) and gets denominators free via a ones column
appended to V; the reciprocal row is broadcast across partitions with a
rank-1 PE matmul.
"""

import numpy as np

B, S, D, H, DK = 4, 2048, 1024, 16, 64
NCORES = 8
HL = 8            # heads per core
W = HL * DK       # 512: local projection width
P = 128
NIC = D // P      # 8  i-chunks
NOC = W // P      # 4  o-chunks (q/k), each holding 2 heads
NSC = S // 512    # 4  512-wide s-chunks (proj moving dim, attention q-blocks)
NVC = S // P      # 16 128-wide s-chunks (v / out-proj partition chunks)
NMC = S // P      # 16 key chunks per head max
THETA = 10000.0

_CACHE = {}


def _build_nc(reps=1):
    import concourse.mybir as mybir
    import concourse.tile as tile
    from concourse import bacc

    f32 = mybir.dt.float32
    f32r = mybir.dt.float32r
    bf16 = mybir.dt.bfloat16
    Exp = mybir.ActivationFunctionType.Exp

    nc = bacc.Bacc("TRN2", target_bir_lowering=False, debug=False,
                   num_devices=NCORES)

    x_t = nc.dram_tensor("x_t", [D, S], bf16, kind="ExternalInput").ap()
    wq_t = nc.dram_tensor("wq_t", [D, W], bf16, kind="ExternalInput").ap()
    wk_t = nc.dram_tensor("wk_t", [D, W], bf16, kind="ExternalInput").ap()
    wv_t = nc.dram_tensor("wv_t", [D, W], bf16, kind="ExternalInput").ap()
    wo_t = nc.dram_tensor("wo_t", [W, D], bf16, kind="ExternalInput").ap()
    cos_t = nc.dram_tensor("cos_t", [P, S], f32, kind="ExternalInput").ap()
    sin_t = nc.dram_tensor("sin_t", [P, S], f32, kind="ExternalInput").ap()
    mask_t = nc.dram_tensor("mask_t", [P, P], bf16, kind="ExternalInput").ap()
    out = nc.dram_tensor("out", [S, D], bf16, kind="ExternalOutput").ap()

    with tile.TileContext(nc) as tc:
        with (
            tc.tile_pool(name="consts", bufs=1) as cpool,
            tc.tile_pool(name="stage", bufs=3) as spool,
            tc.tile_pool(name="psum", bufs=4, space="PSUM") as ppool,
        ):
          for _rep in range(reps):
            # ---- persistent SBUF tensors ----
            xT = [cpool.tile([P, S], bf16, tag=f"xT{i}", name=f"xT{i}")
                  for i in range(NIC)]
            wq = [cpool.tile([P, W], bf16, tag=f"wq{i}", name=f"wq{i}")
                  for i in range(NIC)]
            wk = [cpool.tile([P, W], bf16, tag=f"wk{i}", name=f"wk{i}")
                  for i in range(NIC)]
            wv = [cpool.tile([P, W], bf16, tag=f"wv{i}", name=f"wv{i}")
                  for i in range(NIC)]
            wo = [cpool.tile([P, D], bf16, tag=f"wo{i}", name=f"wo{i}")
                  for i in range(NOC)]
            cos = cpool.tile([P, S], f32, tag="cos", name="cos")
            sin = cpool.tile([P, S], f32, tag="sin", name="sin")
            msk = cpool.tile([P, P], bf16, tag="msk", name="msk")
            ones_r = cpool.tile([1, DK], bf16, tag="ones_r", name="ones_r")
            qT = [cpool.tile([P, S], bf16, tag=f"qT{i}", name=f"qT{i}")
                  for i in range(NOC)]
            kT = [cpool.tile([P, S], bf16, tag=f"kT{i}", name=f"kT{i}")
                  for i in range(NOC)]
            vp = cpool.tile([P, NVC, HL, DK + 1], bf16, tag="vp", name="vp")
            oT = [cpool.tile([P, S], bf16, tag=f"oT{i}", name=f"oT{i}")
                  for i in range(NOC)]

            nc.vector.memset(vp[:, :, :, DK:DK + 1], 1.0)
            nc.vector.memset(ones_r, 1.0)

            # ---- load inputs (already bf16 from host) ----
            # interleaved per i-chunk so projection accumulation can start
            # before all loads land
            for i in range(NIC):
                nc.sync.dma_start(out=xT[i], in_=x_t[i * P:(i + 1) * P, :])
                for wsb, wdr in ((wv, wv_t), (wq, wq_t), (wk, wk_t)):
                    nc.sync.dma_start(out=wsb[i], in_=wdr[i * P:(i + 1) * P, :])
            for i in range(NOC):
                nc.sync.dma_start(out=wo[i], in_=wo_t[i * P:(i + 1) * P, :])
            nc.sync.dma_start(out=cos, in_=cos_t)
            nc.sync.dma_start(out=sin, in_=sin_t)
            nc.sync.dma_start(out=msk, in_=mask_t)

            # ---- QKV projections (+ RoPE on q, k) ----
            for sc in range(NVC):
                pv = ppool.tile([P, 512], f32, tag="mm", name="pv", bufs=2)
                for i in range(NIC):
                    nc.tensor.matmul(
                        pv, xT[i][:, sc * P:(sc + 1) * P], wv[i],
                        start=(i == 0), stop=(i == NIC - 1))
                nc.scalar.copy(
                    out=vp[:, sc, :, 0:DK],
                    in_=pv.rearrange("p (h d) -> p h d", h=HL))

            def emit_qk_proj(wsb, dst, oc):
                    for sc in range(NSC):
                        pj = ppool.tile([P, 512], f32, tag="mm", name="pj", bufs=2)
                        for i in range(NIC):
                            nc.tensor.matmul(
                                pj, wsb[i][:, oc * P:(oc + 1) * P],
                                xT[i][:, sc * 512:(sc + 1) * 512],
                                start=(i == 0), stop=(i == NIC - 1))
                        qsb = spool.tile([P, 512], bf16, tag="qsb", name="qsb", bufs=4)
                        nc.scalar.copy(out=qsb, in_=pj)
                        swp = spool.tile([P, 512], bf16, tag="swp", name="swp", bufs=4)
                        for a, b_ in ((0, 32), (32, 0), (64, 96), (96, 64)):
                            nc.sync.dma_start(out=swp[a:a + 32, :],
                                              in_=qsb[b_:b_ + 32, :])
                        ra = spool.tile([P, 512], f32, tag="ra", name="ra", bufs=3)
                        nc.vector.tensor_mul(ra, pj, cos[:, sc * 512:(sc + 1) * 512])
                        rb = spool.tile([P, 512], f32, tag="rb", name="rb", bufs=3)
                        nc.vector.tensor_mul(rb, swp, sin[:, sc * 512:(sc + 1) * 512])
                        nc.gpsimd.tensor_add(
                            out=dst[oc][:, sc * 512:(sc + 1) * 512],
                            in0=ra, in1=rb)

            # ---- attention ----
            # Chunks fully below the diagonal use the whole 512-query block;
            # the 4 diagonal chunks of each (h, j) only touch queries
            # >= 128*t, so score/exp/AV all restrict to those columns and the
            # triangle mask shrinks to one [128, 128] pattern.
            def emit_attention(h, j):
                    koff = (h % 2) * DK
                    ktile = kT[h // 2]
                    qtile = qT[h // 2]
                    nmc = 4 * (j + 1)
                    qcols = slice(j * 512, (j + 1) * 512)
                    av = ppool.tile([P, 512], f32, tag="mm", name="av",
                                    bufs=2)
                    for pr in range(2 * j):        # full chunk pairs
                        c0 = 2 * pr
                        stg = ppool.tile([P, 2, 512], f32, tag="b2",
                                         name="stg", bufs=3)
                        for u in (0, 1):
                            nc.tensor.matmul(
                                stg[:, u, :],
                                ktile[koff:koff + DK,
                                      (c0 + u) * P:(c0 + u + 1) * P],
                                qtile[koff:koff + DK, qcols],
                                start=True, stop=True)
                        pT = spool.tile([P, 2, 512], bf16, tag="pT", name="pT",
                                        bufs=4)
                        nc.scalar.activation(out=pT, in_=stg, func=Exp,
                                             scale=0.125)
                        for u in (0, 1):
                            nc.tensor.matmul(
                                av[0:DK + 1, :], vp[:, c0 + u, h, 0:DK + 1],
                                pT[:, u, :],
                                start=(c0 + u == 0), stop=False)
                    for t in range(4):             # diagonal chunks
                        c = 4 * j + t
                        col0 = 128 * t
                        wdt = 512 - col0
                        stg = ppool.tile([P, 2, 512], f32, tag="b2",
                                         name="std", bufs=3)
                        stg = stg[:, 0, :]
                        nc.tensor.matmul(
                            stg[:, 0:wdt],
                            ktile[koff:koff + DK, c * P:(c + 1) * P],
                            qtile[koff:koff + DK,
                                  j * 512 + col0:(j + 1) * 512],
                            start=True, stop=True)
                        pT = spool.tile([P, 2, 512], bf16, tag="pT", name="pT",
                                        bufs=4)
                        nc.scalar.activation(out=pT[:, 0, 0:wdt],
                                             in_=stg[:, 0:wdt], func=Exp,
                                             scale=0.125)
                        nc.vector.tensor_mul(pT[:, 0, 0:P], pT[:, 0, 0:P], msk)
                        nc.tensor.matmul(
                            av[0:DK + 1, col0:512], vp[:, c, h, 0:DK + 1],
                            pT[:, 0, 0:wdt],
                            start=(c == 0), stop=(c == nmc - 1))
                    recip = spool.tile([1, 512], bf16, tag="recip",
                                       name="recip", bufs=2)
                    with nc.allow_low_precision(reason="bf16 denominators"):
                        nc.vector.reciprocal(recip, av[DK:DK + 1, :])
                    rbp = ppool.tile([P, 512], f32, tag="mm", name="rbp",
                                     bufs=2)
                    nc.tensor.matmul(rbp[0:DK, :], ones_r, recip,
                                     start=True, stop=True)
                    rbs = spool.tile([DK, 512], f32, tag="rbs", name="rbs")
                    nc.vector.tensor_copy(out=rbs, in_=rbp[0:DK, :])
                    nc.vector.tensor_mul(
                        out=oT[h // 2][koff:koff + DK, j * 512:(j + 1) * 512],
                        in0=av[0:DK, :], in1=rbs)

            for oc in range(NOC):
                emit_qk_proj(wq, qT, oc)
                emit_qk_proj(wk, kT, oc)

            # ---- output projection ----
            def emit_outproj(sc):
                for on in range(2):
                    po = ppool.tile([P, 512], f32, tag="mm", name="po", bufs=2)
                    for dc in range(NOC):
                        nc.tensor.matmul(
                            po, oT[dc][:, sc * P:(sc + 1) * P],
                            wo[dc][:, on * 512:(on + 1) * 512],
                            start=(dc == 0), stop=(dc == NOC - 1))
                    ost = spool.tile([P, 512], bf16, tag="ost", name="ost")
                    nc.vector.tensor_copy(out=ost, in_=po)
                    nc.sync.dma_start(
                        out=out[sc * P:(sc + 1) * P, on * 512:(on + 1) * 512],
                        in_=ost)

            for j in range(NSC):
                for h in range(HL):
                    emit_attention(h, j)
                for sc in range(4 * j, 4 * j + 4):
                    emit_outproj(sc)

    nc.compile()
    return nc


def _host_tables():
    freqs = 1.0 / (THETA ** (np.arange(0, DK, 2, dtype=np.float64) / DK))  # [32]
    t = np.arange(S, dtype=np.float64)
    fm = np.outer(t, freqs)                    # [S, 32]
    pidx = np.arange(P) % 32
    sign = np.where(np.arange(P) % DK < 32, -1.0, 1.0)
    cos_rep = np.cos(fm)[:, pidx].T.astype(np.float32)          # [128, S]
    sin_rep = (np.sin(fm)[:, pidx] * sign[None, :]).T.astype(np.float32)
    cos_rep = np.ascontiguousarray(cos_rep)
    sin_rep = np.ascontiguousarray(sin_rep)

    swap = np.where(np.arange(P) % DK < 32, np.arange(P) + 32, np.arange(P) - 32)
    perm_np = np.zeros((P, P), np.float32)
    perm_np[np.arange(P), swap] = 1.0

    mask_np = (np.arange(P)[:, None] <= np.arange(P)[None, :]).astype(np.float32)
    return cos_rep, sin_rep, perm_np, mask_np


def kernel(x, Wq, Wk, Wv, Wo):
    from concourse.bass_utils import run_bass_kernel_spmd

    if "nc" not in _CACHE:
        _CACHE["nc"] = _build_nc()
    nc = _CACHE["nc"]

    in_maps = build_in_maps(x, Wq, Wk, Wv, Wo)
    res = run_bass_kernel_spmd(nc, in_maps, core_ids=list(range(NCORES)))
    out = np.empty((B, S, D), np.float32)
    for b in range(B):
        out[b] = (res.results[2 * b]["out"].astype(np.float32)
                  + res.results[2 * b + 1]["out"].astype(np.float32))
    return out


def build_in_maps(x, Wq, Wk, Wv, Wo):
    import ml_dtypes
    bf = ml_dtypes.bfloat16
    x = np.asarray(x, np.float32)
    Wq, Wk, Wv, Wo = (np.asarray(w, np.float32) for w in (Wq, Wk, Wv, Wo))
    cos_rep, sin_rep, perm_np, mask_np = _host_tables()
    mask_bf = mask_np.astype(bf)

    # de-interleave RoPE pairs inside each head's 64 rows
    d = np.arange(DK)
    rope_order = np.concatenate([2 * d[:32], 2 * d[:32] + 1])   # [0,2,..,1,3,..]

    in_maps = []
    for c in range(NCORES):
        b, g = divmod(c, 2)
        rows = (np.arange(W) // DK + g * HL)[:, None] * DK  # head base per row
        qk_rows = (rows + rope_order[np.arange(W) % DK][:, None]).ravel()
        v_rows = g * W + np.arange(W)
        in_maps.append({
            "x_t": np.ascontiguousarray(x[b].T.astype(bf)),
            "wq_t": np.ascontiguousarray(Wq[qk_rows, :].T.astype(bf)),
            "wk_t": np.ascontiguousarray(Wk[qk_rows, :].T.astype(bf)),
            "wv_t": np.ascontiguousarray(Wv[v_rows, :].T.astype(bf)),
            "wo_t": np.ascontiguousarray(Wo[:, v_rows].T.astype(bf)),
            "cos_t": cos_rep, "sin_t": sin_rep, "mask_t": mask_bf,
        })
    return in_maps

